# revision 1
# baseline (speedup 1.0000x reference)
"""V5d: data-parallel wavefront, two-group combined elementwise.

On top of V5's statically-addressed recurrence matmuls (27ns/MM bursts), the
gate elementwise is combined across layer groups {L1,L3} and {L2,L4} to cut
the DVE op count per step from ~24 tiny ops to 4 z-adds + 2x6 wide ops,
while keeping two independent chains so the engines stagger. c state bf16.
Gate relu runs on DVE (off Act), h history copies on GpSimd (off-chain).
"""

import sys

sys.path.insert(0, "/opt/trn_rl_repo")

import numpy as np
import ml_dtypes

import concourse.bass as bass
import concourse.bacc as bacc
import concourse.mybir as mybir
import concourse.tile as tile
import time as _time
from concourse.bass_utils import run_bass_kernel_spmd

F32 = mybir.dt.float32
BF16 = mybir.dt.bfloat16
AF = mybir.ActivationFunctionType

B, T, INPUT_LEN = 64, 1024, 256
NCORES = 8
BC = B // NCORES
TC = 64
NCH = T // TC
NL = 4
LAYERS = [(256, 256), (256, 128), (128, 256), (256, 256)]
KHS = [h // 128 for _, h in LAYERS]
NPH = NCH + NL - 1
UNROLL = 16

# two chain groups: layers {0, 2} and {1, 3}
GROUPS = [[0, 2], [1, 3]]
# offset of each layer's kh tiles within its group's tile dim
GOFF = {}
GNT = []
for gi, g in enumerate(GROUPS):
    o = 0
    for li in g:
        GOFF[li] = (gi, o)
        o += KHS[li]
    GNT.append(o)

_CACHE = {}


def _gate_perm(h):
    return np.concatenate(
        [np.arange(0, h), np.arange(h, 2 * h), np.arange(3 * h, 4 * h), np.arange(2 * h, 3 * h)]
    )


def _fold_w(w):
    k, n = w.shape
    kt = k // 128
    return np.ascontiguousarray(w.reshape(kt, 128, n).transpose(1, 0, 2).reshape(128, kt * n))


def _build():
    nc = bacc.Bacc("TRN2", target_bir_lowering=False, debug=False, num_devices=NCORES)

    xT_d = nc.dram_tensor("xT", [128, 2, T, BC], BF16, kind="ExternalInput")
    out_d = nc.dram_tensor("outT", [128, 2, T, BC], BF16, kind="ExternalOutput")
    w_d, u_d, b_d = [], [], []
    for li, (f, h) in enumerate(LAYERS):
        kf, kh, m = f // 128, h // 128, 4 * h // 128
        w_d.append(nc.dram_tensor(f"W{li}", [128, kf * 4 * h], BF16, kind="ExternalInput"))
        u_d.append(nc.dram_tensor(f"U{li}", [128, kh * 4 * h], BF16, kind="ExternalInput"))
        b_d.append(nc.dram_tensor(f"b{li}", [128, m], F32, kind="ExternalInput"))

    with tile.TileContext(nc) as tc:
        with (
            tc.tile_pool(name="const", bufs=1) as cpool,
            tc.tile_pool(name="state", bufs=1) as spool,
            tc.tile_pool(name="xin", bufs=2) as xpool,
            tc.tile_pool(name="zpsum", bufs=1, space="PSUM") as zpp,
            tc.tile_pool(name="ipsum", bufs=2, space="PSUM") as ipp,
        ):
            w_sb, u_sb, b_sb, zx_sb, hist_sb, zps = [], [], [], [], [], []
            for li, (f, h) in enumerate(LAYERS):
                kf, kh, m = f // 128, h // 128, 4 * h // 128
                w_sb.append(cpool.tile([128, kf * 4 * h], BF16, tag=f"w{li}", name=f"w{li}"))
                u_sb.append(cpool.tile([128, kh * 4 * h], BF16, tag=f"u{li}", name=f"u{li}"))
                b_sb.append(cpool.tile([128, m], F32, tag=f"b{li}", name=f"b{li}"))
                nc.sync.dma_start(w_sb[li][:], w_d[li][:])
                nc.sync.dma_start(u_sb[li][:], u_d[li][:])
                nc.sync.dma_start(b_sb[li][:], b_d[li][:])
                zx_sb.append(
                    spool.tile([128, 4, kh, TC, BC], BF16, tag=f"zx{li}", name=f"zx{li}")
                )
                hist_sb.append(
                    spool.tile([128, kh, TC, BC], BF16, tag=f"hist{li}", name=f"hist{li}")
                )
                nb = 2 if li in (0, 2) else 1
                zps.append(
                    [
                        zpp.tile([128, 4, kh, 1, BC], F32, tag=f"zp{li}_{q}", name=f"zp{li}_{q}")
                        for q in range(nb)
                    ]
                )

            # group-combined tiles: [128, kind(4), nt, 1, BC]
            z_g, g_g, c_g, t1_g, t2_g, hc_g = [], [], [], [], [], []
            for gi in range(2):
                nt = GNT[gi]
                z_g.append(spool.tile([128, 4, nt, 1, BC], BF16, tag=f"zg{gi}", name=f"zg{gi}"))
                g_g.append(spool.tile([128, 4, nt, 1, BC], BF16, tag=f"gg{gi}", name=f"gg{gi}"))
                c_g.append(spool.tile([128, nt, 1, BC], BF16, tag=f"cg{gi}", name=f"cg{gi}"))
                t1_g.append(spool.tile([128, nt, 1, BC], BF16, tag=f"t1g{gi}", name=f"t1g{gi}"))
                t2_g.append(spool.tile([128, nt, 1, BC], BF16, tag=f"t2g{gi}", name=f"t2g{gi}"))
                # parity-buffered current h per group (static matmul source)
                hc_g.append(
                    spool.tile([128, 2, nt, 1, BC], BF16, tag=f"hcg{gi}", name=f"hcg{gi}")
                )
            for li in range(NL):
                for t in zps[li]:
                    nc.vector.memset(t[:], 0.0)
            for gi in range(2):
                nc.vector.memset(hc_g[gi][:], 0.0)
                nc.vector.memset(c_g[gi][:], 0.0)

            def step_mms(li, par):
                f, h = LAYERS[li]
                kh = h // 128
                fh = 4 * h
                gi, o = GOFF[li]
                zt = zps[li][par % len(zps[li])]
                for mi in range(4 * kh):
                    kind, j = mi // kh, mi % kh
                    for k in range(kh):
                        nc.tensor.matmul(
                            zt[:, kind, j, :, :],
                            u_sb[li][:, k * fh + mi * 128 : k * fh + (mi + 1) * 128],
                            hc_g[gi][:, par, o + k, :, :],
                            start=(k == 0),
                            stop=(k == kh - 1),
                        )

            def z_add(li, iv, par):
                kh = KHS[li]
                gi, o = GOFF[li]
                nc.vector.tensor_add(
                    z_g[gi][:, :, o : o + kh, :, :],
                    zps[li][par % len(zps[li])][:],
                    zx_sb[li][:, :, :, bass.ds(iv, 1), :],
                )

            def grp_act(gi):
                nc.scalar.activation(g_g[gi][:, 0:3], z_g[gi][:, 0:3], AF.Sigmoid)
                nc.vector.tensor_scalar_max(g_g[gi][:, 3], z_g[gi][:, 3], 0.0)

            def step_elem(active, iv, par):
                for gi in range(2):
                    i_k, f_k, o_k, gk = (
                        g_g[gi][:, 0],
                        g_g[gi][:, 1],
                        g_g[gi][:, 2],
                        g_g[gi][:, 3],
                    )
                    nc.vector.tensor_mul(t1_g[gi][:], i_k, gk)
                    nc.vector.tensor_mul(t2_g[gi][:], f_k, c_g[gi][:])
                    nc.vector.tensor_add(c_g[gi][:], t1_g[gi][:], t2_g[gi][:])
                    nc.vector.tensor_mul(t2_g[gi][:], o_k, c_g[gi][:])
                    nc.vector.tensor_scalar_max(hc_g[gi][:, 1 - par], t2_g[gi][:], 0.0)
                # off-chain: record h into per-layer history
                for li in active:
                    kh = KHS[li]
                    gi, o = GOFF[li]
                    nc.gpsimd.tensor_copy(
                        hist_sb[li][:, :, bass.ds(iv, 1), :],
                        hc_g[gi][:, 1 - par, o : o + kh, :, :],
                    )

            def inproj(li, src):
                f, h = LAYERS[li]
                kf, kh, m = f // 128, h // 128, 4 * h // 128
                fh = 4 * h
                for mi in range(m):
                    kind, j = mi // kh, mi % kh
                    ps = ipp.tile([128, TC, BC], F32, tag="ip", name="ip")
                    for k in range(kf):
                        nc.tensor.matmul(
                            ps[:],
                            w_sb[li][:, k * fh + mi * 128 : k * fh + (mi + 1) * 128],
                            src[:, k, :, :],
                            start=(k == 0),
                            stop=(k == kf - 1),
                        )
                    nc.scalar.activation(
                        zx_sb[li][:, kind, j, :, :],
                        ps[:],
                        AF.Identity,
                        bias=b_sb[li][:, mi : mi + 1],
                    )

            for p in range(NPH):
                active = [li for li in range(NL) if 0 <= p - li < NCH]
                for li in active:
                    c = p - li
                    if li == 0:
                        xt = xpool.tile([128, 2, TC, BC], BF16, tag="xt", name="xt")
                        nc.sync.dma_start(xt[:], xT_d[:, :, c * TC : (c + 1) * TC, :])
                        inproj(0, xt)
                    else:
                        inproj(li, hist_sb[li - 1])
                    if c == 0:
                        gi, o = GOFF[li]
                        kh = KHS[li]
                        nc.vector.memset(hc_g[gi][:, 0, o : o + kh, :, :], 0.0)
                        nc.vector.memset(c_g[gi][:, o : o + kh, :, :], 0.0)
                with tc.For_i(0, TC, UNROLL) as iv:
                    for u in range(UNROLL):
                        # group-major: G1's activations start while PE runs G2
                        for gi in range(2):
                            for li in GROUPS[gi]:
                                if li in active:
                                    step_mms(li, u % 2)
                                    z_add(li, iv + u, u % 2)
                            grp_act(gi)
                        step_elem(active, iv + u, u % 2)
                if NL - 1 in active:
                    c4 = p - (NL - 1)
                    nc.sync.dma_start(
                        out_d[:, :, c4 * TC : (c4 + 1) * TC, :], hist_sb[NL - 1][:]
                    )
    nc.compile()
    return nc


def _prep_inputs(x, ws, us, bs):
    base = {}
    for li, (f, h) in enumerate(LAYERS):
        perm = _gate_perm(h)
        base[f"W{li}"] = _fold_w(ws[li][:, perm]).astype(ml_dtypes.bfloat16)
        base[f"U{li}"] = _fold_w(us[li][:, perm]).astype(ml_dtypes.bfloat16)
        bb = bs[li][perm]
        base[f"b{li}"] = np.ascontiguousarray(bb.reshape(4 * h // 128, 128).T)

    in_maps = []
    for ci in range(NCORES):
        xc = x[ci * BC : (ci + 1) * BC]
        xT = np.ascontiguousarray(xc.reshape(BC, T, 2, 128).transpose(3, 2, 1, 0)).astype(
            ml_dtypes.bfloat16
        )
        m = dict(base)
        m["xT"] = xT
        in_maps.append(m)
    return in_maps


def kernel(x, W1, U1, b1, W2, U2, b2, W3, U3, b3, W4, U4, b4):
    x = np.asarray(x, dtype=np.float32)
    ws = [np.asarray(a, np.float32) for a in (W1, W2, W3, W4)]
    us = [np.asarray(a, np.float32) for a in (U1, U2, U3, U4)]
    bs = [np.asarray(a, np.float32) for a in (b1, b2, b3, b4)]

    if "nc" not in _CACHE:
        _CACHE["nc"] = _build()
    nc = _CACHE["nc"]

    in_maps = _prep_inputs(x, ws, us, bs)
    _CACHE["last_in_maps"] = in_maps

    res = None
    last_err = None
    for _attempt in range(3):
        try:
            res = run_bass_kernel_spmd(nc, in_maps, list(range(NCORES)))
            break
        except Exception as e:  # transient device-unrecoverable reports
            last_err = e
            _time.sleep(5)
    if res is None:
        raise last_err
    outs = []
    for ci in range(NCORES):
        oT = np.asarray(res.results[ci]["outT"], dtype=np.float32)
        outs.append(np.ascontiguousarray(oT.transpose(3, 2, 1, 0).reshape(BC, T, 256)))
    return np.concatenate(outs, axis=0)

